# revision 9
# baseline (speedup 1.0000x reference)
"""GATv2 encoder (2-layer, PyG GATv2Conv semantics) on 8 TRN2 NeuronCores.

Sharding: dst-node blocks of 6250 nodes per core; edges live with their dst
core so segment softmax/aggregation are local; one AllGather of the folded
source-side node table between layers.

Algorithm (host-validated against the jax reference to ~5e-6 rel err):
- |att| folded into Wl/Wr columns, columns permuted pos-att-first per head.
  Per-edge logits become  sum_pos lrelu(u) - sum_neg lrelu(u)  with
  u = ul[src] + ur[dst] gathered directly from folded tables (second gather
  accumulates via the DMA CCE-add path).
- Segment softmax skips max-subtraction (|logits| <= ~1 for this model).
- sum_e alpha*(ul+ur) = sum_e alpha*ul + ur, so the same u tiles feed the
  aggregation; per-chunk one-hot matmul accumulates [num | den] in PSUM.
"""
import numpy as np

try:
    import concourse  # noqa: F401
except ImportError:  # pragma: no cover
    import sys
    sys.path.insert(0, "/opt/trn_rl_repo")

from concourse import bass, bacc, mybir, tile
from concourse import bass_utils
from concourse.bass import IndirectOffsetOnAxis

F32 = mybir.dt.float32
I32 = mybir.dt.int32

N_NODES = 50000
N_CORES = 8
FEAT = 128
HEADS1 = 4


class Cfg:
    def __init__(self, n_nodes, n_cores, feat, heads1, T, dtype=F32):
        self.N = n_nodes
        self.NC = n_cores
        self.NPC = n_nodes // n_cores
        self.P = 128
        self.CHUNKS = (self.NPC + 127) // 128
        self.SLOTS = self.CHUNKS * 128
        self.F = feat
        self.H1 = heads1
        self.T = T
        self.TD = dtype


# ---------------------------------------------------------------- host prep

def prep_weights(att, Wl, bl, Wr, br, bias):
    H, C = att.shape
    a = att.reshape(-1).astype(np.float64)
    perm, pos_counts = [], []
    for h in range(H):
        cols = np.arange(h * C, (h + 1) * C)
        pos = cols[a[cols] >= 0]
        neg = cols[a[cols] < 0]
        perm.extend(pos.tolist() + neg.tolist())
        pos_counts.append(len(pos))
    perm = np.array(perm, dtype=np.int64)
    absa = np.maximum(np.abs(a[perm]), 1e-12)
    return dict(
        perm=perm, pos_counts=pos_counts,
        Wl=(Wl[:, perm] * absa[None, :]).astype(np.float32),
        bl=(bl[perm] * absa).astype(np.float32),
        Wr=(Wr[:, perm] * absa[None, :]).astype(np.float32),
        br=(br[perm] * absa).astype(np.float32),
        inva=(1.0 / absa).astype(np.float32),
        bias=bias[perm].astype(np.float32),
    )


def prep_graph(edge_index, cfg, T_override=None):
    """Per-core chunked edge layout for dma_gather (int16 indices).

    Edges of each chunk are ordered [src<32768 section | src>=32768 section],
    each section padded to a global fixed tile count (T_LO / T_HI).  Gather
    index arrays are int16, wrapped in 16 partitions (column-major groups of
    16) and replicated 8x down the partition dim as the HW requires.
    Pads: src->row 0 of its half-table, dst-table->SLOTS (zeroed dummy row),
    slot->999 (no one-hot match), node_ids pad->SLOTS+8.
    """
    import heapq
    N, NPC, P, CHUNKS = cfg.N, cfg.NPC, cfg.P, cfg.CHUNKS
    HALF = 32768
    src = np.asarray(edge_index[0], dtype=np.int64)
    dst = np.asarray(edge_index[1], dtype=np.int64)
    loops = np.arange(N, dtype=np.int64)
    src = np.concatenate([src, loops])
    dst = np.concatenate([dst, loops])

    cores = []
    maxTlo = maxThi = 0
    for c in range(cfg.NC):
        lo = c * NPC
        m = (dst >= lo) & (dst < lo + NPC)
        s_c = src[m]
        d_c = dst[m] - lo
        deg = np.bincount(d_c, minlength=NPC)
        order = np.argsort(-deg, kind="stable")
        heap = [(0, g) for g in range(CHUNKS)]
        heapq.heapify(heap)
        bin_nodes = [[] for _ in range(CHUNKS)]
        bin_sum = [0] * CHUNKS
        for n in order:
            while True:
                sm, g = heapq.heappop(heap)
                if len(bin_nodes[g]) < P:
                    break
            bin_nodes[g].append(int(n))
            bin_sum[g] = sm + int(deg[n])
            if len(bin_nodes[g]) < P:
                heapq.heappush(heap, (bin_sum[g], g))
        eorder = np.argsort(d_c, kind="stable")
        starts = np.zeros(NPC + 1, dtype=np.int64)
        np.cumsum(deg, out=starts[1:])
        s_sorted = s_c[eorder]
        # per-chunk edge lists split by src half
        chunk_edges = []
        for g in range(CHUNKS):
            lo_s, lo_d, hi_s, hi_d = [], [], [], []
            for slot, n in enumerate(bin_nodes[g]):
                a, b = starts[n], starts[n + 1]
                for s_val in s_sorted[a:b]:
                    if s_val < HALF:
                        lo_s.append(s_val); lo_d.append((n, slot))
                    else:
                        hi_s.append(s_val - HALF); hi_d.append((n, slot))
            maxTlo = max(maxTlo, (len(lo_s) + P - 1) // P)
            maxThi = max(maxThi, (len(hi_s) + P - 1) // P)
            chunk_edges.append((lo_s, lo_d, hi_s, hi_d))
        cores.append((bin_nodes, chunk_edges))

    T_LO = max(maxTlo, 1)
    T_HI = max(maxThi, 1) if N > HALF else maxThi
    T = T_LO + T_HI

    def wrap16(ids):
        # position i -> unwrapped[i]; wrapped[p, s] = ids[s*16 + p]; tile 8x
        a = np.asarray(ids, dtype=np.int16).reshape(-1, 16).T
        return np.tile(a, (8, 1))

    out = []
    for c in range(cfg.NC):
        bin_nodes, chunk_edges = cores[c]
        xlw = np.zeros((CHUNKS, 128, T * 8), dtype=np.int16)
        xrw = np.zeros((CHUNKS, 128, T * 8), dtype=np.int16)
        dstl = np.full((CHUNKS, P, T), 999.0, dtype=np.float32)
        node_ids = np.full((CHUNKS, P), cfg.SLOTS + 8, dtype=np.int32)
        for g in range(CHUNKS):
            lo_s, lo_d, hi_s, hi_d = chunk_edges[g]
            for slot, n in enumerate(bin_nodes[g]):
                node_ids[g, slot] = n
            n_lo, n_hi = T_LO * P, T_HI * P
            ls = np.zeros(n_lo, np.int64); ls[:len(lo_s)] = lo_s
            hs = np.zeros(n_hi, np.int64); hs[:len(hi_s)] = hi_s
            xd = np.full(n_lo + n_hi, cfg.SLOTS, np.int64)
            sl = np.full(n_lo + n_hi, 999.0, np.float32)
            for j, (n, slot) in enumerate(lo_d):
                xd[j] = n; sl[j] = slot
            for j, (n, slot) in enumerate(hi_d):
                xd[n_lo + j] = n; sl[n_lo + j] = slot
            xlw[g, :, :T_LO * 8] = wrap16(ls)
            xlw[g, :, T_LO * 8:] = wrap16(hs)
            xrw[g] = wrap16(xd)
            # position i -> (t=i//128, p=i%128)
            dstl[g] = sl.reshape(T, P).T
        out.append(dict(xlw=xlw, xrw=xrw, dstl=dstl, node_ids=node_ids))
    return out, (T, T_LO, T_HI)


def make_core_inputs(core_id, x, w1, w2, gr, cfg):
    NPC, SLOTS, F = cfg.NPC, cfg.SLOTS, cfg.F
    xb = np.zeros((SLOTS, F), np.float32)
    xb[:NPC] = x[core_id * NPC:(core_id + 1) * NPC]
    rowb = lambda v: np.broadcast_to(v.astype(np.float32), (128, F)).copy()
    return {
        "xT_own": np.ascontiguousarray(xb.T),
        "W1l": w1["Wl"], "W1r": w1["Wr"], "W2l": w2["Wl"], "W2r": w2["Wr"],
        "bb1l": rowb(w1["bl"]), "bb1r": rowb(w1["br"]),
        "bb2l": rowb(w2["bl"]), "bb2r": rowb(w2["br"]),
        "inva1": rowb(w1["inva"]), "gbias1": rowb(w1["bias"]),
        "inva2": rowb(w2["inva"]), "gbias2": rowb(w2["bias"]),
        "iotab": np.broadcast_to(np.arange(128, dtype=np.float32), (128, 128)).copy(),
        "ident": np.eye(128, dtype=np.float32),
        "xlw": gr["xlw"], "xrw": gr["xrw"], "dstl": gr["dstl"],
        "node_ids": gr["node_ids"],
    }


# ---------------------------------------------------------------- device

def declare_io(nc, cfg):
    CH, P, T, F, SLOTS = cfg.CHUNKS, cfg.P, cfg.T, cfg.F, cfg.SLOTS
    d = {}
    def inp(name, shape, dt=F32):
        d[name] = nc.dram_tensor(name, list(shape), dt, kind="ExternalInput").ap()
    inp("xT_own", (F, SLOTS))
    for n in ("W1l", "W1r", "W2l", "W2r", "bb1l", "bb1r", "bb2l", "bb2r",
              "inva1", "gbias1", "inva2", "gbias2", "iotab", "ident"):
        inp(n, (128, F))
    inp("xlw", (CH, P, T * 8), mybir.dt.int16)
    inp("xrw", (CH, P, T * 8), mybir.dt.int16)
    inp("dstl", (CH, P, T), F32)
    inp("node_ids", (CH, P), I32)
    d["out"] = nc.dram_tensor("out", [SLOTS, F], F32, kind="ExternalOutput").ap()
    return d


def build_program(tc, io, cfg, pos_counts1, pos_counts2):
    nc = tc.nc
    P, F, T, CH = cfg.P, cfg.F, cfg.T, cfg.CHUNKS
    NPC, SLOTS, TD = cfg.NPC, cfg.SLOTS, cfg.TD
    N = cfg.N

    with (
        tc.tile_pool(name="consts", bufs=1) as cpool,
        tc.tile_pool(name="work", bufs=2) as wp,
        tc.tile_pool(name="small", bufs=3) as sp,
        tc.tile_pool(name="psum", bufs=2, space="PSUM") as pp,
        tc.tile_pool(name="dram", bufs=1, space="DRAM") as dp,
    ):
        C = {}
        for n in ("W1l", "W1r", "W2l", "W2r"):
            t = cpool.tile([128, F], TD, tag=n)
            nc.sync.dma_start(t[:], io[n])
            C[n] = t
        for n in ("bb1l", "bb1r", "bb2l", "bb2r", "inva1", "gbias1",
                  "inva2", "gbias2", "iotab"):
            t = cpool.tile([128, F], F32, tag=n)
            nc.sync.dma_start(t[:], io[n])
            C[n] = t
        ident = cpool.tile([128, 128], TD, tag="ident")
        nc.sync.dma_start(ident[:], io["ident"])
        zeros = cpool.tile([128, F], TD, tag="zeros")
        nc.vector.memset(zeros[:], 0.0)

        xl_own = dp.tile([SLOTS, F], TD)
        xr_own = dp.tile([SLOTS + 16, F], TD)
        ag_space = "Shared" if cfg.NC > 4 else "Local"
        xl_full = dp.tile([N, F], TD, addr_space=ag_space)
        h_block = dp.tile([SLOTS + 16, F], TD)
        hl_own = dp.tile([SLOTS, F], TD)
        hr_own = dp.tile([SLOTS + 16, F], TD)
        hl_full = dp.tile([N, F], TD, addr_space=ag_space)

        for tab in (xr_own, hr_own, h_block):
            nc.sync.dma_start(tab[SLOTS:SLOTS + 16, :], zeros[0:16, :])
        if SLOTS > NPC:
            nc.sync.dma_start(h_block[NPC:SLOTS, :], zeros[0:SLOTS - NPC, :])

        for g in range(CH):
            xT_sb = sp.tile([128, 128], TD, tag="xT")
            nc.sync.dma_start(xT_sb[:], io["xT_own"][:, g * 128:(g + 1) * 128])
            ps_l = pp.tile([128, F], F32, tag="tab_l")
            ps_r = pp.tile([128, F], F32, tag="tab_r")
            nc.tensor.matmul(ps_l[:], lhsT=xT_sb[:], rhs=C["W1l"][:], start=True, stop=True)
            nc.tensor.matmul(ps_r[:], lhsT=xT_sb[:], rhs=C["W1r"][:], start=True, stop=True)
            xl_sb = sp.tile([128, F], TD, tag="xl_sb")
            xr_sb = sp.tile([128, F], TD, tag="xr_sb")
            nc.vector.tensor_tensor(out=xl_sb[:], in0=ps_l[:], in1=C["bb1l"][:], op=mybir.AluOpType.add)
            nc.vector.tensor_tensor(out=xr_sb[:], in0=ps_r[:], in1=C["bb1r"][:], op=mybir.AluOpType.add)
            nc.sync.dma_start(xl_own[g * 128:(g + 1) * 128, :], xl_sb[:])
            nc.sync.dma_start(xr_own[g * 128:(g + 1) * 128, :], xr_sb[:])

        nc.gpsimd.collective_compute(
            "AllGather", mybir.AluOpType.bypass,
            replica_groups=[list(range(cfg.NC))],
            ins=[xl_own[0:NPC, :]], outs=[xl_full[:, :]],
        )

        def edge_layer(tab_full, tab_own, H, pos_counts, inva, gbias, elu, out_to):
            Ch = F // H
            for g in range(CH):
                TLO, THI = cfg.T_LO, cfg.T_HI
                HALF = 32768
                xlw_sb = sp.tile([P, T * 8], mybir.dt.int16, tag="xlw")
                xrw_sb = sp.tile([P, T * 8], mybir.dt.int16, tag="xrw")
                dstl_sb = sp.tile([P, T], F32, tag="dstl")
                nid_sb = sp.tile([P, 1], I32, tag="nid")
                nc.sync.dma_start(xlw_sb[:], io["xlw"][g])
                nc.sync.dma_start(xrw_sb[:], io["xrw"][g])
                nc.sync.dma_start(dstl_sb[:], io["dstl"][g])
                nc.sync.dma_start(nid_sb[:], io["node_ids"][g].rearrange("(p o) -> p o", o=1))

                MAXT = 8  # <=1024 idxs per dma_gather (ring capacity)

                def gathers(out3, in_ap, idx_sb, t0, t1):
                    for a in range(t0, t1, MAXT):
                        b = min(a + MAXT, t1)
                        nc.gpsimd.dma_gather(
                            out_ap=out3[:, a:b, :], in_ap=in_ap,
                            idxs_ap=idx_sb[:, a * 8:b * 8],
                            num_idxs=(b - a) * P, num_idxs_reg=(b - a) * P,
                            elem_size=F)

                ul = wp.tile([P, T * F], TD, tag="ul")
                ul3 = ul[:].rearrange("p (t f) -> p t f", f=F)
                gathers(ul3, tab_full[0:min(HALF, N), :], xlw_sb, 0, TLO)
                if THI > 0:
                    gathers(ul3, tab_full[HALF:N, :], xlw_sb, TLO, T)
                ure = wp.tile([P, T * F], TD, tag="ure")
                ure3 = ure[:].rearrange("p (t f) -> p t f", f=F)
                gathers(ure3, tab_own[:, :], xrw_sb, 0, T)
                ub = wp.tile([P, T * F], TD, tag="ub")
                nc.vector.tensor_tensor(out=ub[:], in0=ul[:], in1=ure[:],
                                        op=mybir.AluOpType.add)

                lr = wp.tile([P, T * F], TD, tag="lr")
                nc.gpsimd.tensor_scalar(out=lr[:], in0=ub[:], scalar1=0.2,
                                        scalar2=None, op0=mybir.AluOpType.mult)
                nc.vector.tensor_tensor(out=lr[:], in0=ub[:], in1=lr[:],
                                        op=mybir.AluOpType.max)

                lr3 = lr[:].rearrange("p (t f) -> p t f", f=F)
                possum = sp.tile([P, T * H], F32, tag="possum")
                negsum = sp.tile([P, T * H], F32, tag="negsum")
                pos3 = possum[:].rearrange("p (t h) -> p t h", h=H)
                neg3 = negsum[:].rearrange("p (t h) -> p t h", h=H)
                for h in range(H):
                    pc = pos_counts[h]
                    s = h * Ch
                    if pc > 0:
                        nc.vector.tensor_reduce(
                            out=pos3[:, :, h:h + 1], in_=lr3[:, :, s:s + pc],
                            axis=mybir.AxisListType.X, op=mybir.AluOpType.add)
                    else:
                        nc.vector.memset(pos3[:, :, h:h + 1], 0.0)
                    if pc < Ch:
                        nc.vector.tensor_reduce(
                            out=neg3[:, :, h:h + 1], in_=lr3[:, :, s + pc:s + Ch],
                            axis=mybir.AxisListType.X, op=mybir.AluOpType.add)
                    else:
                        nc.vector.memset(neg3[:, :, h:h + 1], 0.0)
                logit = sp.tile([P, T * H], F32, tag="logit")
                nc.vector.tensor_tensor(out=logit[:], in0=possum[:], in1=negsum[:],
                                        op=mybir.AluOpType.subtract)

                aug = wp.tile([P, T * (F + H)], TD, tag="aug")
                aug3 = aug[:].rearrange("p (t c) -> p t c", c=F + H)
                nc.scalar.activation(out=aug3[:, :, F:F + H], in_=logit[:],
                                     func=mybir.ActivationFunctionType.Exp)
                ub4 = ub[:].rearrange("p (t h c) -> p t h c", h=H, c=Ch)
                aug4 = aug3[:, :, 0:F].rearrange("p t (h c) -> p t h c", h=H)
                wb = aug3[:, :, F:F + H].to_broadcast([P, T, H, Ch])
                nc.vector.tensor_tensor(out=aug4, in0=ub4, in1=wb,
                                        op=mybir.AluOpType.mult)

                ps = pp.tile([128, F + H], F32, tag="agg")
                for t in range(T):
                    oh = sp.tile([P, 128], TD, tag="oh")
                    nc.vector.tensor_scalar(
                        out=oh[:], in0=C["iotab"][:], scalar1=dstl_sb[:, t:t + 1],
                        scalar2=None, op0=mybir.AluOpType.is_equal)
                    nc.tensor.matmul(ps[:], lhsT=oh[:],
                                     rhs=aug3[:, t, :],
                                     start=(t == 0), stop=(t == T - 1))

                den = sp.tile([P, H], F32, tag="den")
                nc.vector.tensor_scalar(out=den[:], in0=ps[:, F:F + H],
                                        scalar1=1e-30, scalar2=None,
                                        op0=mybir.AluOpType.add)
                rec = sp.tile([P, H], F32, tag="rec")
                nc.vector.reciprocal(rec[:], den[:])
                urt = sp.tile([P, F], TD, tag="urt")
                nc.gpsimd.indirect_dma_start(
                    out=urt[:], out_offset=None, in_=tab_own[:, :],
                    in_offset=IndirectOffsetOnAxis(ap=nid_sb[:, 0:1], axis=0))
                o1 = sp.tile([P, F], F32, tag="o1")
                if H > 1:
                    nc.vector.tensor_tensor(
                        out=o1[:].rearrange("p (h c) -> p h c", h=H),
                        in0=ps[:, 0:F].rearrange("p (h c) -> p h c", h=H),
                        in1=rec[:].to_broadcast([P, H, Ch]),
                        op=mybir.AluOpType.mult)
                else:
                    nc.vector.tensor_scalar(out=o1[:], in0=ps[:, 0:F],
                                            scalar1=rec[:, 0:1], scalar2=None,
                                            op0=mybir.AluOpType.mult)
                if TD != F32:
                    urf = sp.tile([P, F], F32, tag="urf")
                    nc.vector.tensor_copy(out=urf[:], in_=urt[:])
                else:
                    urf = urt
                nc.vector.tensor_tensor(out=o1[:], in0=o1[:], in1=urf[:],
                                        op=mybir.AluOpType.subtract)
                nc.vector.tensor_tensor(out=o1[:], in0=o1[:], in1=inva[:],
                                        op=mybir.AluOpType.mult)
                nc.vector.tensor_tensor(out=o1[:], in0=o1[:], in1=gbias[:],
                                        op=mybir.AluOpType.add)
                if elu:
                    m0 = sp.tile([P, F], F32, tag="m0")
                    nc.vector.tensor_scalar(out=m0[:], in0=o1[:], scalar1=0.0,
                                            scalar2=None, op0=mybir.AluOpType.min)
                    e0 = sp.tile([P, F], F32, tag="e0")
                    nc.scalar.activation(out=e0[:], in_=m0[:],
                                         func=mybir.ActivationFunctionType.Exp)
                    nc.vector.tensor_scalar(out=o1[:], in0=o1[:], scalar1=0.0,
                                            scalar2=None, op0=mybir.AluOpType.max)
                    nc.vector.tensor_tensor(out=o1[:], in0=o1[:], in1=e0[:],
                                            op=mybir.AluOpType.add)
                    nc.vector.tensor_scalar(out=o1[:], in0=o1[:], scalar1=1.0,
                                            scalar2=None, op0=mybir.AluOpType.subtract)
                if out_to == "h_block":
                    if TD != F32:
                        hcast = sp.tile([P, F], TD, tag="hcast")
                        nc.vector.tensor_copy(out=hcast[:], in_=o1[:])
                        src_tile = hcast
                    else:
                        src_tile = o1
                    nc.gpsimd.indirect_dma_start(
                        out=h_block[:, :],
                        out_offset=IndirectOffsetOnAxis(ap=nid_sb[:, 0:1], axis=0),
                        in_=src_tile[:], in_offset=None)
                else:
                    nc.sync.dma_start(io["out"][g * 128:(g + 1) * 128, :], o1[:])

        edge_layer(xl_full, xr_own, cfg.H1, pos_counts1,
                   C["inva1"], C["gbias1"], elu=True, out_to="h_block")

        for g in range(CH):
            h_sb = sp.tile([128, F], TD, tag="h_sb")
            nc.sync.dma_start(h_sb[:], h_block[g * 128:(g + 1) * 128, :])
            ps_t = pp.tile([128, 128], F32, tag="tr")
            nc.tensor.transpose(out=ps_t[:], in_=h_sb[:], identity=ident[:])
            hT_sb = sp.tile([128, 128], TD, tag="hT")
            nc.vector.tensor_copy(out=hT_sb[:], in_=ps_t[:])
            ps_l = pp.tile([128, F], F32, tag="tab_l")
            ps_r = pp.tile([128, F], F32, tag="tab_r")
            nc.tensor.matmul(ps_l[:], lhsT=hT_sb[:], rhs=C["W2l"][:], start=True, stop=True)
            nc.tensor.matmul(ps_r[:], lhsT=hT_sb[:], rhs=C["W2r"][:], start=True, stop=True)
            hl_sb = sp.tile([128, F], TD, tag="xl_sb")
            hr_sb = sp.tile([128, F], TD, tag="xr_sb")
            nc.vector.tensor_tensor(out=hl_sb[:], in0=ps_l[:], in1=C["bb2l"][:], op=mybir.AluOpType.add)
            nc.vector.tensor_tensor(out=hr_sb[:], in0=ps_r[:], in1=C["bb2r"][:], op=mybir.AluOpType.add)
            nc.sync.dma_start(hl_own[g * 128:(g + 1) * 128, :], hl_sb[:])
            nc.sync.dma_start(hr_own[g * 128:(g + 1) * 128, :], hr_sb[:])

        nc.gpsimd.collective_compute(
            "AllGather", mybir.AluOpType.bypass,
            replica_groups=[list(range(cfg.NC))],
            ins=[hl_own[0:NPC, :]], outs=[hl_full[:, :]],
        )

        edge_layer(hl_full, hr_own, 1, pos_counts2,
                   C["inva2"], C["gbias2"], elu=False, out_to="out")


# ---------------------------------------------------------------- runner

_LAST = {}


def kernel(**inputs) -> np.ndarray:
    x = np.asarray(inputs["x"], np.float32)
    ei = np.asarray(inputs["edge_index"])
    w1 = prep_weights(np.asarray(inputs["att1"], np.float32),
                      np.asarray(inputs["W1l"], np.float32),
                      np.asarray(inputs["b1l"], np.float32),
                      np.asarray(inputs["W1r"], np.float32),
                      np.asarray(inputs["b1r"], np.float32),
                      np.asarray(inputs["bias1"], np.float32))
    w2 = prep_weights(np.asarray(inputs["att2"], np.float32),
                      np.asarray(inputs["W2l"], np.float32)[w1["perm"], :],
                      np.asarray(inputs["b2l"], np.float32),
                      np.asarray(inputs["W2r"], np.float32)[w1["perm"], :],
                      np.asarray(inputs["b2r"], np.float32),
                      np.asarray(inputs["bias2"], np.float32))
    cfg = Cfg(N_NODES, N_CORES, FEAT, HEADS1, T=None)
    grs, (T, T_LO, T_HI) = prep_graph(ei, cfg)
    cfg.T, cfg.T_LO, cfg.T_HI = T, T_LO, T_HI

    in_maps = [make_core_inputs(c, x, w1, w2, grs[c], cfg) for c in range(N_CORES)]

    nc = bacc.Bacc("TRN2", target_bir_lowering=False, debug=False,
                   num_devices=N_CORES)
    io = declare_io(nc, cfg)
    with tile.TileContext(nc) as tc:
        build_program(tc, io, cfg, w1["pos_counts"], w2["pos_counts"])
    nc.compile()

    res = bass_utils.run_bass_kernel_spmd(nc, in_maps, core_ids=list(range(N_CORES)))
    _LAST["results"] = res
    _LAST["nc"] = nc
    _LAST["in_maps"] = in_maps
    _LAST["cfg"] = cfg

    out = np.zeros((cfg.N, cfg.F), np.float32)
    for c in range(N_CORES):
        oc = np.asarray(res.results[c]["out"])
        ni = grs[c]["node_ids"].ravel()
        valid = ni < cfg.NPC
        out[c * cfg.NPC + ni[valid]] = oc.reshape(cfg.SLOTS, cfg.F)[valid]
    final = np.empty_like(out)
    final[:, w2["perm"]] = out
    return final
